# revision 36
# baseline (speedup 1.0000x reference)
"""Trainium2 Bass kernel for a 4-layer binary MLP (BinaryFCNN).

Reference computation (per layer):  h = sign_pm1(h @ sign_pm1(W).T + b)
with x: [8192, 4096] fp32, W_l: [4096, 4096] fp32, b_l: [4096] fp32.

Strategy (v2)
-------------
* Data-parallel over the batch: core c processes rows [c*1024, (c+1)*1024).
  No collectives.
* Layer 1 is the only inexact matmul: x is split ON HOST into fp16 digits
  hi = fp16(x), lo = fp16(x - hi) (lo relies on fp16 subnormals, which the
  PE honors exactly -- probed). Both digit passes share the SAME +-0.5 fp16
  stationary weights and accumulate into one PSUM bank, so the pair needs
  one weight load. Effective precision ~23 bits; a single borderline sign
  flip vs the fp32 reference is the irreducible level.
* Layer-1 loop is ko-outer over nb-groups-of-4 with 4 concurrent PSUM banks
  and chunk-streamed digits/weights, so the PE starts ~4us into the kernel
  (no separate warmup burst; the first ~3.4us of real matmuls run at the
  cold 1.2 GHz HAM clock, costing ~1.7us once).
* Layers 2..4 are bit-exact: +-1 activations (fp8) x +-0.5 weights
  (host-encoded fp8 planes, half the DMA of raw bf16) with fp8 DoubleRow
  (2 MACs/PE/cycle) and fp32 PSUM accumulation. The kp-loop is outermost
  with both batch-half PSUM banks accumulating in parallel, so consecutive
  matmuls share the stationary operand and a post-scheduling pass deletes
  the redundant 256-column LDWEIGHTS.
* ACT applies Sign(2*psum + b) with the per-feature bias as the per-partition
  bias operand; outputs land feature-major (split per DR kp-pair for
  range-accurate cross-layer deps) so each layer's output is directly the
  next layer's moving operand.
* DMAs are few and large (fused hi+lo digit blocks, 1MB weight transfers
  covering 4 output blocks) and emission-ordered so ~1.5MB in 2 transfers
  gates the first matmul; a ~3us warmup burst on a memset tile covers the
  gate and releases the HAM clock gate (cold 1.2 GHz -> warm 2.4 GHz).

Measured on 8 axon-tunneled TRN2 NeuronCores: HW exec ~1.568 ms (from the
1.620 ms baseline), relative error 0.0092 (708/33.5M element mismatches = 1
borderline layer-1 sign flip amplified through the binary net -- the
irreducible level for non-bit-identical fp32 arithmetic; identical across
runs). PE-busy density is ~99.8%: zero stream gaps >150ns; residual overhead
is ~8us runtime prologue, ~3.5us warmup, ~9us end-of-kernel barrier cascade,
and ~3ns/MM DoubleRow issue cadence.
"""
import numpy as np
import ml_dtypes

import concourse.bass as bass
import concourse.tile as tile
from concourse import bacc
import concourse.mybir as mybir
from concourse.bass_utils import run_bass_kernel_spmd

F32 = mybir.dt.float32
F16 = mybir.dt.float16
BF16 = mybir.dt.bfloat16
FP8 = mybir.dt.float8e4
SIGN = mybir.ActivationFunctionType.Sign
DR = mybir.MatmulPerfMode.DoubleRow

N_CORES = 8
D_FULL = 4096
B_FULL = 8192
MF = 512          # moving free dim == one fp32 PSUM bank
KOI = 8           # digit ko-chunks per SBUF block
GRP = 4           # nb group size (concurrent PSUM banks in layer 1)


def build_binary_mlp(D: int, M: int, n_layers: int = 4) -> bass.Bass:
    KO = D // 128
    NB = D // 128
    MH = M // MF
    KOB = KO // KOI   # digit blocks per mh
    NG = NB // GRP    # nb groups in layer 1
    KQ = KO // 4      # w1 quarter size in ko chunks

    nc = bacc.Bacc("TRN2", target_bir_lowering=False, debug=False)
    # hi/lo digit planes interleaved: one DMA per block covers both digits
    xdig = nc.declare_dram_parameter("xdig", [MH, KOB, 128, KOI, 2, MF], F16,
                                     isOutput=False)
    # w1 packed so one DMA covers a whole (group, quarter): [g, q, k, j, koq, n]
    w1d = nc.declare_dram_parameter("w1", [NG, 4, 128, GRP, KQ, 128], F16,
                                    isOutput=False)
    wds = [
        nc.declare_dram_parameter(f"w{l + 1}", [NB, 128, KO, 128], FP8, isOutput=False)
        for l in range(1, n_layers)
    ]
    bd = nc.declare_dram_parameter("bias", [128, n_layers, NB], F32, isOutput=False)
    out = nc.declare_dram_parameter("out", [NB, 128, M], BF16, isOutput=True)

    with tile.TileContext(nc) as tc:
        with (
            tc.tile_pool(name="const", bufs=1) as const,
            tc.tile_pool(name="dig", bufs=1) as dig,
            tc.tile_pool(name="w1p", bufs=4) as w1p,
            tc.tile_pool(name="w8p", bufs=4) as w8p,
            tc.tile_pool(name="oio", bufs=4) as oio,
            tc.tile_pool(name="psum", bufs=8, space="PSUM") as psum,
        ):
            # ping-pong activation buffers, feature-major, +-1 in fp8.
            # Split per DR kp-pair so cross-layer deps are range-accurate
            # (a monolithic tile would serialize each layer's first matmul
            # on the previous layer's LAST activation).
            hAt = [const.tile([128, 2, M], FP8, tag=f"hA{t}", name=f"hA{t}")
                   for t in range(KO // 2)]
            hBt = [const.tile([128, 2, M], FP8, tag=f"hB{t}", name=f"hB{t}")
                   for t in range(KO // 2)]

            bt = const.tile([128, n_layers, NB], F32, tag="bias", name="bias")
            nc.sync.dma_start(bt[:], bd[:])
            bias_tiles = [bt[:, l, :] for l in range(n_layers)]

            # HAM warm-up: ~3us of throwaway matmuls gated only on a memset,
            # run while the first digit/weight DMAs stream, so the real
            # stream starts at the warm 2.4 GHz clock.
            wu = const.tile([128, MF], F16, tag="warm", name="warm")
            nc.vector.memset(wu[:], 1.0)
            wps = psum.tile([128, MF], F32, tag="ps", name="wps")
            n_wu = 11
            for i in range(n_wu):
                nc.tensor.matmul(wps[:], wu[:, :128], wu[:],
                                 start=(i == 0), stop=(i == n_wu - 1))

            # ---------------- layer 1: fp16 hi+lo digit passes ----------------
            # ko-outer over groups of GRP output blocks; hi/lo share stationary.
            # DMA emission order matters: the queue drains in program order, so
            # only ~1.5MB in 2 transfers gates the kernel's first MM.
            for mh in range(MH):
                ms = slice(mh * MF, (mh + 1) * MF)
                dgb = [None] * KOB

                # block 0 is split into pieces [2, 2, 4] interleaved with the
                # first weight DMAs
                B0 = ((0, 2), (2, 2), (4, KOI - 4))

                def load_digit_piece(p, mh=mh, dgb=dgb):
                    o, n = B0[p]
                    if p == 0:
                        dgb[0] = []
                    t = dig.tile([128, n, 2, MF], F16, tag=f"dg0{p}", name=f"dg0{p}")
                    nc.sync.dma_start(t[:], xdig[mh, 0, :, o:o + n, :, :])
                    dgb[0].append(t)

                def load_digit_block(b, mh=mh, dgb=dgb):
                    dgb[b] = dig.tile([128, KOI, 2, MF], F16, tag=f"dg{b}", name=f"dg{b}")
                    nc.sync.dma_start(dgb[b][:], xdig[mh, b])

                def digit_slice(b, i, d):
                    if b == 0:
                        for p, (o, n) in enumerate(B0):
                            if i < o + n:
                                return dgb[0][p][:, i - o, d, :]
                    return dgb[b][:, i, d, :]

                load_digit_piece(0)
                for g in range(NG):
                    nbs = [g * GRP + j for j in range(GRP)]
                    wq = []
                    for q in range(4):
                        t = w1p.tile([128, GRP, KQ, 128], F16, tag="w1q",
                                     name=f"w1_{g}_{q}")
                        if g == 0 and q == 0:
                            # split the very first weight transfer so only
                            # ~1MB gates the kernel's first matmul (subtile
                            # deps release the ko 0..3 slices early)
                            nc.sync.dma_start(t[:, :, 0:KQ // 2, :],
                                              w1d[g, q, :, :, 0:KQ // 2, :])
                            nc.sync.dma_start(t[:, :, KQ // 2:KQ, :],
                                              w1d[g, q, :, :, KQ // 2:KQ, :])
                        else:
                            nc.sync.dma_start(t[:], w1d[g, q])
                        wq.append(t)
                        if g == 0:
                            if q == 0:
                                load_digit_piece(1)
                                load_digit_piece(2)
                            if q + 1 < KOB:
                                load_digit_block(q + 1)
                    pss = [psum.tile([128, MF], F32, tag="ps", name=f"ps{g}_{j}")
                           for j in range(GRP)]
                    for ko in range(KO):
                        b, i = divmod(ko, KOI)
                        q, koq = divmod(ko, KQ)
                        hi_mv = digit_slice(b, i, 0)
                        lo_mv = digit_slice(b, i, 1)
                        for j in range(GRP):
                            lhs = wq[q][:, j, koq, :]
                            nc.tensor.matmul(pss[j], lhs, hi_mv,
                                             start=(ko == 0), stop=False)
                            nc.tensor.matmul(pss[j], lhs, lo_mv,
                                             start=False, stop=(ko == KO - 1))
                    for j, nb in enumerate(nbs):
                        # h1 = Sign(2*psum + b)  (psum = 0.5 * x~ @ sign(W).T)
                        nc.scalar.activation(hAt[nb // 2][:, nb % 2, ms], pss[j], SIGN,
                                             bias=bias_tiles[0][:, nb:nb + 1], scale=2.0)

            # ---------------- layers 2..n: exact +-1 x +-0.5 fp8 DR ----------------
            # kp-outer with both mh PSUM banks accumulating in parallel: the two
            # matmuls of a kp step share the stationary operand, so the dedup
            # pass below drops every second (256-column, 213ns) LDWEIGHTS and
            # the weight-load path stops co-saturating with the matmul stream.
            hin, hout = hAt, hBt
            for l in range(1, n_layers):
                last = l == n_layers - 1
                for nb in range(NB):
                    wt = w8p.tile([128, KO, 128], FP8, tag="w8", name=f"w8_{l}_{nb}")
                    nc.sync.dma_start(wt[:], wds[l - 1][nb])
                    pss = [psum.tile([128, MF], F32, tag="ps", name=f"ps{l}_{nb}_{mh}")
                           for mh in range(MH)]
                    for kp in range(0, KO, 2):
                        lhs = wt[:, kp:kp + 2, :]
                        for mh in range(MH):
                            nc.tensor.matmul(
                                pss[mh][:], lhs,
                                hin[kp // 2][:, :, mh * MF:(mh + 1) * MF],
                                start=(kp == 0), stop=(kp + 2 == KO),
                                perf_mode=DR)
                    for mh in range(MH):
                        ms = slice(mh * MF, (mh + 1) * MF)
                        if last:
                            ot = oio.tile([128, MF], BF16, tag="ot", name="ot")
                            nc.scalar.activation(ot[:], pss[mh][:], SIGN,
                                                 bias=bias_tiles[l][:, nb:nb + 1], scale=2.0)
                            # issue on the ACT engine's own HWDGE queue: the
                            # DMA follows its producing ACT in FIFO order with
                            # no cross-engine semaphore hop on the tail path
                            nc.scalar.dma_start(out[nb, :, ms], ot[:])
                        else:
                            nc.scalar.activation(hout[nb // 2][:, nb % 2, ms], pss[mh][:], SIGN,
                                                 bias=bias_tiles[l][:, nb:nb + 1], scale=2.0)
                hin, hout = hout, hin
    _dedup_ldweights(nc)
    nc.compile()
    return nc


def _dedup_ldweights(nc) -> int:
    """Delete an InstLdweights when it re-loads exactly what the previous
    InstLdweights loaded (same AP + perf mode): the PE keeps the stationary
    operand across matmuls, so the reload is pure weight-path traffic. Deps of
    the deleted load are merged into the matmul it was serving."""
    n_del = 0
    for func in nc.m.functions:
        for block in func.blocks:
            insts = list(block.instructions)
            kept = []
            last_key = None
            pending = None
            changed = False
            for inst in insts:
                tn = type(inst).__name__
                if tn == "InstLdweights":
                    ap = inst.ins[0]
                    c = ap.concise() if callable(ap.concise) else ap.concise
                    key = (str(c), str(inst.perf_mode), str(inst.is_transpose))
                    if key == last_key and pending is None:
                        pending = inst
                        changed = True
                        n_del += 1
                        continue
                    last_key = key
                elif tn == "InstMatmult" and pending is not None:
                    inst.merge_dependencies_from(pending)
                    pending = None
                kept.append(inst)
            if changed:
                block.instructions = kept
    return n_del


def _pack_w8(W: np.ndarray, npdt) -> np.ndarray:
    """W [D, D] fp32 -> [NB, 128(p=k_in), KO, 128(n)] in npdt with values
    +-0.5 = 0.5*sign_pm1(W):  WP[nb, p, ko, n] = 0.5*sgn(W[nb*128 + n, ko*128 + p])."""
    D = W.shape[0]
    nb = D // 128
    S = np.where(W >= 0, np.float32(0.5), np.float32(-0.5))
    return np.ascontiguousarray(
        S.astype(npdt).reshape(nb, 128, nb, 128).transpose(0, 3, 2, 1)
    )


def _pack_w1(W: np.ndarray) -> np.ndarray:
    """W [D, D] fp32 -> [NG, 4, 128(k), GRP, KQ, 128(n)] fp16 +-0.5 so one DMA
    covers a whole (nb-group, ko-quarter)."""
    D = W.shape[0]
    NG, KQ = D // (128 * GRP), D // (128 * 4)
    S = np.where(W >= 0, np.float16(0.5), np.float16(-0.5))
    return np.ascontiguousarray(
        S.reshape(NG, GRP, 128, 4, KQ, 128).transpose(0, 3, 5, 1, 4, 2)
    )


def _pack_b(b: np.ndarray) -> np.ndarray:
    return np.ascontiguousarray(b.astype(np.float32).reshape(-1, 128).T)


def _pack_digits(hi: np.ndarray, lo: np.ndarray, M: int) -> np.ndarray:
    """hi/lo [M, D] fp16 -> [MH, KOB, 128, KOI, 2, MF] chunk-contiguous with
    the two digit planes interleaved."""
    D = hi.shape[1]
    MH, KOB = M // MF, D // (KOI * 128)
    arr = np.stack([hi, lo], axis=0).reshape(2, MH, MF, KOB, KOI, 128)
    return np.ascontiguousarray(arr.transpose(1, 3, 5, 4, 0, 2))


last_result = None  # BassKernelResults of the most recent run (for test.py)
_nc_cache = {}


def kernel(x, W1, b1, W2, b2, W3, b3, W4, b4):
    global last_result
    assert x.shape == (B_FULL, D_FULL)
    M = B_FULL // N_CORES

    if (D_FULL, M) not in _nc_cache:
        _nc_cache[(D_FULL, M)] = build_binary_mlp(D_FULL, M)
    nc = _nc_cache[(D_FULL, M)]

    x = np.asarray(x, dtype=np.float32)
    hi = x.astype(np.float16)
    lo = (x - hi.astype(np.float32)).astype(np.float16)

    shared = {"w1": _pack_w1(np.asarray(W1))}
    for l, W in enumerate((W2, W3, W4), start=2):
        shared[f"w{l}"] = _pack_w8(np.asarray(W), ml_dtypes.float8_e4m3)
    shared["bias"] = np.ascontiguousarray(
        np.stack([_pack_b(np.asarray(b)) for b in (b1, b2, b3, b4)], axis=1))

    in_maps = []
    for c in range(N_CORES):
        m = dict(shared)
        m["xdig"] = _pack_digits(hi[c * M:(c + 1) * M], lo[c * M:(c + 1) * M], M)
        in_maps.append(m)

    try:
        res = run_bass_kernel_spmd(nc, in_maps, core_ids=list(range(N_CORES)))
    except Exception:
        # one retry for transient device hiccups
        res = run_bass_kernel_spmd(nc, in_maps, core_ids=list(range(N_CORES)))
    last_result = res

    parts = []
    for c in range(N_CORES):
        o = np.asarray(res.results[c]["out"])  # [NB, 128, M] bf16, values +-1
        parts.append(o.reshape(D_FULL, M).T)   # -> [M, D] (rows are batch)
    return np.concatenate(parts, axis=0).astype(np.float32)


# revision 38
# speedup vs baseline: 1.0007x; 1.0007x over previous
"""Trainium2 Bass kernel for a 4-layer binary MLP (BinaryFCNN).

Reference computation (per layer):  h = sign_pm1(h @ sign_pm1(W).T + b)
with x: [8192, 4096] fp32, W_l: [4096, 4096] fp32, b_l: [4096] fp32.

Strategy (v2)
-------------
* Data-parallel over the batch: core c processes rows [c*1024, (c+1)*1024).
  No collectives.
* Layer 1 is the only inexact matmul: x is split ON HOST into fp16 digits
  hi = fp16(x), lo = fp16(x - hi) (lo relies on fp16 subnormals, which the
  PE honors exactly -- probed). Both digit passes share the SAME +-0.5 fp16
  stationary weights and accumulate into one PSUM bank, so the pair needs
  one weight load. Effective precision ~23 bits; a single borderline sign
  flip vs the fp32 reference is the irreducible level.
* Layer-1 loop is ko-outer over nb-groups-of-4 with 4 concurrent PSUM banks
  and chunk-streamed digits/weights, so the PE starts ~4us into the kernel
  (no separate warmup burst; the first ~3.4us of real matmuls run at the
  cold 1.2 GHz HAM clock, costing ~1.7us once).
* Layers 2..4 are bit-exact: +-1 activations (fp8) x +-0.5 weights
  (host-encoded fp8 planes, half the DMA of raw bf16) with fp8 DoubleRow
  (2 MACs/PE/cycle) and fp32 PSUM accumulation. The kp-loop is outermost
  with both batch-half PSUM banks accumulating in parallel, so consecutive
  matmuls share the stationary operand and a post-scheduling pass deletes
  the redundant 256-column LDWEIGHTS.
* ACT applies Sign(2*psum + b) with the per-feature bias as the per-partition
  bias operand; outputs land feature-major (split per DR kp-pair for
  range-accurate cross-layer deps) so each layer's output is directly the
  next layer's moving operand.
* DMAs are few and large (fused hi+lo digit blocks, 1MB weight transfers
  covering 4 output blocks) and emission-ordered so ~1.5MB in 2 transfers
  gates the first matmul; a ~3us warmup burst on a memset tile covers the
  gate and releases the HAM clock gate (cold 1.2 GHz -> warm 2.4 GHz).

Measured on 8 axon-tunneled TRN2 NeuronCores: HW exec ~1.568 ms (from the
1.620 ms baseline), relative error 0.0092 (708/33.5M element mismatches = 1
borderline layer-1 sign flip amplified through the binary net -- the
irreducible level for non-bit-identical fp32 arithmetic; identical across
runs). PE-busy density is ~99.8%: zero stream gaps >150ns; residual overhead
is ~8us runtime prologue, ~3.5us warmup, ~9us end-of-kernel barrier cascade,
and ~3ns/MM DoubleRow issue cadence.
"""
import numpy as np
import ml_dtypes

import concourse.bass as bass
import concourse.tile as tile
from concourse import bacc
import concourse.mybir as mybir
from concourse.bass_utils import run_bass_kernel_spmd

F32 = mybir.dt.float32
F16 = mybir.dt.float16
BF16 = mybir.dt.bfloat16
FP8 = mybir.dt.float8e4
SIGN = mybir.ActivationFunctionType.Sign
DR = mybir.MatmulPerfMode.DoubleRow

N_CORES = 8
D_FULL = 4096
B_FULL = 8192
MF = 512          # moving free dim == one fp32 PSUM bank
KOI = 8           # digit ko-chunks per SBUF block
GRP = 4           # nb group size (concurrent PSUM banks in layer 1)


def build_binary_mlp(D: int, M: int, n_layers: int = 4) -> bass.Bass:
    KO = D // 128
    NB = D // 128
    MH = M // MF
    KOB = KO // KOI   # digit blocks per mh
    NG = NB // GRP    # nb groups in layer 1
    KQ = KO // 4      # w1 quarter size in ko chunks

    nc = bacc.Bacc("TRN2", target_bir_lowering=False, debug=False)
    # hi/lo digit planes interleaved: one DMA per block covers both digits
    xdig = nc.declare_dram_parameter("xdig", [MH, KOB, 128, KOI, 2, MF], F16,
                                     isOutput=False)
    # w1 packed so one DMA covers a whole (group, quarter): [g, q, k, j, koq, n]
    w1d = nc.declare_dram_parameter("w1", [NG, 4, 128, GRP, KQ, 128], F16,
                                    isOutput=False)
    wds = [
        nc.declare_dram_parameter(f"w{l + 1}", [NB, 128, KO, 128], FP8, isOutput=False)
        for l in range(1, n_layers)
    ]
    bd = nc.declare_dram_parameter("bias", [128, n_layers, NB], F32, isOutput=False)
    out = nc.declare_dram_parameter("out", [NB, 128, M], BF16, isOutput=True)

    with tile.TileContext(nc) as tc:
        with (
            tc.tile_pool(name="const", bufs=1) as const,
            tc.tile_pool(name="dig", bufs=1) as dig,
            tc.tile_pool(name="w1p", bufs=4) as w1p,
            tc.tile_pool(name="w8p", bufs=4) as w8p,
            tc.tile_pool(name="oio", bufs=4) as oio,
            tc.tile_pool(name="psum", bufs=8, space="PSUM") as psum,
        ):
            # ping-pong activation buffers, feature-major, +-1 in fp8.
            # Split per DR kp-pair so cross-layer deps are range-accurate
            # (a monolithic tile would serialize each layer's first matmul
            # on the previous layer's LAST activation).
            hAt = [const.tile([128, 2, M], FP8, tag=f"hA{t}", name=f"hA{t}")
                   for t in range(KO // 2)]
            hBt = [const.tile([128, 2, M], FP8, tag=f"hB{t}", name=f"hB{t}")
                   for t in range(KO // 2)]

            # bias tile is created here but its DMA is emitted after the
            # layer-1 gate parcel (it is only needed at the first ACT, ~65us
            # in, and must not sit ahead of the gate in the queue)
            bt = const.tile([128, n_layers, NB], F32, tag="bias", name="bias")
            bias_tiles = [bt[:, l, :] for l in range(n_layers)]
            bias_loaded = [False]

            # HAM warm-up: ~3us of throwaway matmuls gated only on a memset,
            # run while the first digit/weight DMAs stream, so the real
            # stream starts at the warm 2.4 GHz clock.
            wu = const.tile([128, MF], F16, tag="warm", name="warm")
            nc.vector.memset(wu[:], 1.0)
            wps = psum.tile([128, MF], F32, tag="ps", name="wps")
            n_wu = 11
            for i in range(n_wu):
                nc.tensor.matmul(wps[:], wu[:, :128], wu[:],
                                 start=(i == 0), stop=(i == n_wu - 1))

            # ---------------- layer 1: fp16 hi+lo digit passes ----------------
            # ko-outer over groups of GRP output blocks; hi/lo share stationary.
            # DMA emission order matters: the queue drains in program order, so
            # only ~1.5MB in 2 transfers gates the kernel's first MM.
            for mh in range(MH):
                ms = slice(mh * MF, (mh + 1) * MF)
                dgb = [None] * KOB

                # block 0 is split into pieces [2, 2, 4] interleaved with the
                # first weight DMAs
                B0 = ((0, 2), (2, 2), (4, KOI - 4))

                def load_digit_piece(p, mh=mh, dgb=dgb):
                    o, n = B0[p]
                    if p == 0:
                        dgb[0] = []
                    t = dig.tile([128, n, 2, MF], F16, tag=f"dg0{p}", name=f"dg0{p}")
                    nc.sync.dma_start(t[:], xdig[mh, 0, :, o:o + n, :, :])
                    dgb[0].append(t)

                def load_digit_block(b, mh=mh, dgb=dgb):
                    dgb[b] = dig.tile([128, KOI, 2, MF], F16, tag=f"dg{b}", name=f"dg{b}")
                    nc.sync.dma_start(dgb[b][:], xdig[mh, b])

                def digit_slice(b, i, d):
                    if b == 0:
                        for p, (o, n) in enumerate(B0):
                            if i < o + n:
                                return dgb[0][p][:, i - o, d, :]
                    return dgb[b][:, i, d, :]

                load_digit_piece(0)
                for g in range(NG):
                    nbs = [g * GRP + j for j in range(GRP)]
                    wq = []
                    for q in range(4):
                        t = w1p.tile([128, GRP, KQ, 128], F16, tag="w1q",
                                     name=f"w1_{g}_{q}")
                        if g == 0 and q == 0:
                            # split the very first weight transfer so only
                            # ~1MB gates the kernel's first matmul (subtile
                            # deps release the ko 0..3 slices early)
                            nc.sync.dma_start(t[:, :, 0:KQ // 2, :],
                                              w1d[g, q, :, :, 0:KQ // 2, :])
                            nc.sync.dma_start(t[:, :, KQ // 2:KQ, :],
                                              w1d[g, q, :, :, KQ // 2:KQ, :])
                        else:
                            nc.sync.dma_start(t[:], w1d[g, q])
                        wq.append(t)
                        if g == 0:
                            if q == 0:
                                if not bias_loaded[0]:
                                    nc.sync.dma_start(bt[:], bd[:])
                                    bias_loaded[0] = True
                                load_digit_piece(1)
                                load_digit_piece(2)
                            if q + 1 < KOB:
                                load_digit_block(q + 1)
                    pss = [psum.tile([128, MF], F32, tag="ps", name=f"ps{g}_{j}")
                           for j in range(GRP)]
                    for ko in range(KO):
                        b, i = divmod(ko, KOI)
                        q, koq = divmod(ko, KQ)
                        hi_mv = digit_slice(b, i, 0)
                        lo_mv = digit_slice(b, i, 1)
                        for j in range(GRP):
                            lhs = wq[q][:, j, koq, :]
                            nc.tensor.matmul(pss[j], lhs, hi_mv,
                                             start=(ko == 0), stop=False)
                            nc.tensor.matmul(pss[j], lhs, lo_mv,
                                             start=False, stop=(ko == KO - 1))
                    for j, nb in enumerate(nbs):
                        # h1 = Sign(2*psum + b)  (psum = 0.5 * x~ @ sign(W).T)
                        nc.scalar.activation(hAt[nb // 2][:, nb % 2, ms], pss[j], SIGN,
                                             bias=bias_tiles[0][:, nb:nb + 1], scale=2.0)

            # ---------------- layers 2..n: exact +-1 x +-0.5 fp8 DR ----------------
            # kp-outer with both mh PSUM banks accumulating in parallel: the two
            # matmuls of a kp step share the stationary operand, so the dedup
            # pass below drops every second (256-column, 213ns) LDWEIGHTS and
            # the weight-load path stops co-saturating with the matmul stream.
            hin, hout = hAt, hBt
            for l in range(1, n_layers):
                last = l == n_layers - 1
                for nb in range(NB):
                    wt = w8p.tile([128, KO, 128], FP8, tag="w8", name=f"w8_{l}_{nb}")
                    nc.sync.dma_start(wt[:], wds[l - 1][nb])
                    pss = [psum.tile([128, MF], F32, tag="ps", name=f"ps{l}_{nb}_{mh}")
                           for mh in range(MH)]
                    for kp in range(0, KO, 2):
                        lhs = wt[:, kp:kp + 2, :]
                        for mh in range(MH):
                            nc.tensor.matmul(
                                pss[mh][:], lhs,
                                hin[kp // 2][:, :, mh * MF:(mh + 1) * MF],
                                start=(kp == 0), stop=(kp + 2 == KO),
                                perf_mode=DR)
                    for mh in range(MH):
                        ms = slice(mh * MF, (mh + 1) * MF)
                        if last:
                            ot = oio.tile([128, MF], BF16, tag="ot", name="ot")
                            nc.scalar.activation(ot[:], pss[mh][:], SIGN,
                                                 bias=bias_tiles[l][:, nb:nb + 1], scale=2.0)
                            # issue on the ACT engine's own HWDGE queue: the
                            # DMA follows its producing ACT in FIFO order with
                            # no cross-engine semaphore hop on the tail path
                            nc.scalar.dma_start(out[nb, :, ms], ot[:])
                        else:
                            nc.scalar.activation(hout[nb // 2][:, nb % 2, ms], pss[mh][:], SIGN,
                                                 bias=bias_tiles[l][:, nb:nb + 1], scale=2.0)
                hin, hout = hout, hin
    _dedup_ldweights(nc)
    nc.compile()
    return nc


def _dedup_ldweights(nc) -> int:
    """Delete an InstLdweights when it re-loads exactly what the previous
    InstLdweights loaded (same AP + perf mode): the PE keeps the stationary
    operand across matmuls, so the reload is pure weight-path traffic. Deps of
    the deleted load are merged into the matmul it was serving."""
    n_del = 0
    for func in nc.m.functions:
        for block in func.blocks:
            insts = list(block.instructions)
            kept = []
            last_key = None
            pending = None
            changed = False
            for inst in insts:
                tn = type(inst).__name__
                if tn == "InstLdweights":
                    ap = inst.ins[0]
                    c = ap.concise() if callable(ap.concise) else ap.concise
                    key = (str(c), str(inst.perf_mode), str(inst.is_transpose))
                    if key == last_key and pending is None:
                        pending = inst
                        changed = True
                        n_del += 1
                        continue
                    last_key = key
                elif tn == "InstMatmult" and pending is not None:
                    inst.merge_dependencies_from(pending)
                    pending = None
                kept.append(inst)
            if changed:
                block.instructions = kept
    return n_del


def _pack_w8(W: np.ndarray, npdt) -> np.ndarray:
    """W [D, D] fp32 -> [NB, 128(p=k_in), KO, 128(n)] in npdt with values
    +-0.5 = 0.5*sign_pm1(W):  WP[nb, p, ko, n] = 0.5*sgn(W[nb*128 + n, ko*128 + p])."""
    D = W.shape[0]
    nb = D // 128
    S = np.where(W >= 0, np.float32(0.5), np.float32(-0.5))
    return np.ascontiguousarray(
        S.astype(npdt).reshape(nb, 128, nb, 128).transpose(0, 3, 2, 1)
    )


def _pack_w1(W: np.ndarray) -> np.ndarray:
    """W [D, D] fp32 -> [NG, 4, 128(k), GRP, KQ, 128(n)] fp16 +-0.5 so one DMA
    covers a whole (nb-group, ko-quarter)."""
    D = W.shape[0]
    NG, KQ = D // (128 * GRP), D // (128 * 4)
    S = np.where(W >= 0, np.float16(0.5), np.float16(-0.5))
    return np.ascontiguousarray(
        S.reshape(NG, GRP, 128, 4, KQ, 128).transpose(0, 3, 5, 1, 4, 2)
    )


def _pack_b(b: np.ndarray) -> np.ndarray:
    return np.ascontiguousarray(b.astype(np.float32).reshape(-1, 128).T)


def _pack_digits(hi: np.ndarray, lo: np.ndarray, M: int) -> np.ndarray:
    """hi/lo [M, D] fp16 -> [MH, KOB, 128, KOI, 2, MF] chunk-contiguous with
    the two digit planes interleaved."""
    D = hi.shape[1]
    MH, KOB = M // MF, D // (KOI * 128)
    arr = np.stack([hi, lo], axis=0).reshape(2, MH, MF, KOB, KOI, 128)
    return np.ascontiguousarray(arr.transpose(1, 3, 5, 4, 0, 2))


last_result = None  # BassKernelResults of the most recent run (for test.py)
_nc_cache = {}


def kernel(x, W1, b1, W2, b2, W3, b3, W4, b4):
    global last_result
    assert x.shape == (B_FULL, D_FULL)
    M = B_FULL // N_CORES

    if (D_FULL, M) not in _nc_cache:
        _nc_cache[(D_FULL, M)] = build_binary_mlp(D_FULL, M)
    nc = _nc_cache[(D_FULL, M)]

    x = np.asarray(x, dtype=np.float32)
    hi = x.astype(np.float16)
    lo = (x - hi.astype(np.float32)).astype(np.float16)

    shared = {"w1": _pack_w1(np.asarray(W1))}
    for l, W in enumerate((W2, W3, W4), start=2):
        shared[f"w{l}"] = _pack_w8(np.asarray(W), ml_dtypes.float8_e4m3)
    shared["bias"] = np.ascontiguousarray(
        np.stack([_pack_b(np.asarray(b)) for b in (b1, b2, b3, b4)], axis=1))

    in_maps = []
    for c in range(N_CORES):
        m = dict(shared)
        m["xdig"] = _pack_digits(hi[c * M:(c + 1) * M], lo[c * M:(c + 1) * M], M)
        in_maps.append(m)

    try:
        res = run_bass_kernel_spmd(nc, in_maps, core_ids=list(range(N_CORES)))
    except Exception:
        # one retry for transient device hiccups
        res = run_bass_kernel_spmd(nc, in_maps, core_ids=list(range(N_CORES)))
    last_result = res

    parts = []
    for c in range(N_CORES):
        o = np.asarray(res.results[c]["out"])  # [NB, 128, M] bf16, values +-1
        parts.append(o.reshape(D_FULL, M).T)   # -> [M, D] (rows are batch)
    return np.concatenate(parts, axis=0).astype(np.float32)
